# revision 4
# baseline (speedup 1.0000x reference)
"""Trainium2 Bass kernel for nn_DigitConvolutionalModel.

Model: x[B,784] -> conv3x3(valid, 28x28->26x26) -> flatten -> Linear(676,256)
       -> relu -> Linear(256,10).

The conv is linear, so it is folded into the first Linear on the host:
  h_pre = x @ W1eff + b1,  W1eff[784,256] = C @ W1.T  (C = conv as matrix)
leaving a plain 2-layer MLP for the device:
  out = relu(x @ W1eff + b1) @ W2.T + b2

Sharding: pure data parallelism over the batch dim across 8 NeuronCores
(8192 samples/core); weights replicated. Compute in bf16 with fp32 PSUM
accumulation.

Schedule (v2): the 784-dim contraction is split as 6 full 128-row chunks
plus a 16-row tail. The tail is applied via 4 row-tiled matmuls packed
into distinct 32-row groups of the PE array (they run concurrently), so
layer 1 costs 6 N=512 column passes per (group, half) instead of 7.
Layer 2's two 128-row chunks run as two col-tiled matmuls (col groups 0
and 1) that also overlap in the array; their partial sums are merged
with the bias in one DVE scalar_tensor_tensor op. Batch groups are
processed in blocks of 2 (4 layer-1 PSUM banks live per block).
Startup: the first batch group is loaded in 128-column quarters with
strict DMA ring priority so real matmuls start as early as possible;
a short warmup matmul train keeps the PE busy (HAM un-throttle) during
the initial DMA fill.
"""

import sys

if "/opt/trn_rl_repo" not in sys.path:
    sys.path.insert(0, "/opt/trn_rl_repo")

import ml_dtypes
import numpy as np

B = 65536
NCORES = 8
BC = B // NCORES  # 8192 samples per core
P = 128
KM = 6            # full 128-row contraction chunks (rows 0..767)
KT = 16           # tail contraction rows (768..783)
NF1 = 256         # layer-1 output features (2 halves of 128)
NO = 10           # logits
NB = 512          # batch columns per matmul group (one PSUM bank, fp32)
NGRP = BC // NB   # 16 groups per core
NBLK = NGRP // 2  # 8 blocks of 2 groups

_PROG = None


def _build_program():
    import concourse.tile as tile
    from concourse import bacc, mybir

    bf16 = mybir.dt.bfloat16
    f32 = mybir.dt.float32
    Relu = mybir.ActivationFunctionType.Relu
    add = mybir.AluOpType.add

    nc = bacc.Bacc("TRN2", target_bir_lowering=False, debug=False,
                   num_devices=NCORES)
    xm = nc.dram_tensor("xm", [P, NGRP, KM, NB], bf16,
                        kind="ExternalInput").ap()
    xtl = nc.dram_tensor("xtl", [P, NBLK, NB], bf16,
                         kind="ExternalInput").ap()
    w1 = nc.dram_tensor("w1", [P, KM, NF1], bf16, kind="ExternalInput").ap()
    w1t = nc.dram_tensor("w1t", [P, P], bf16, kind="ExternalInput").ap()
    w2 = nc.dram_tensor("w2", [P, 2, NO], bf16, kind="ExternalInput").ap()
    b1 = nc.dram_tensor("b1", [P, 2], f32, kind="ExternalInput").ap()
    b2 = nc.dram_tensor("b2", [NO, 1], f32, kind="ExternalInput").ap()
    out = nc.dram_tensor("out", [NO, BC], f32, kind="ExternalOutput").ap()

    with tile.TileContext(nc) as tc:
        with (
            tc.tile_pool(name="singles", bufs=1) as singles,
            tc.tile_pool(name="xp", bufs=8) as xp,
            tc.tile_pool(name="xtp", bufs=8) as xtp,
            tc.tile_pool(name="hp", bufs=8) as hp,
            tc.tile_pool(name="op", bufs=6) as op,
            tc.tile_pool(name="ps1", bufs=5, space="PSUM") as ps1p,
            tc.tile_pool(name="ps2", bufs=2, space="PSUM") as ps2p,
        ):
            # --- warmup: keep the PE busy from the first possible cycle so
            # the HAM clock gate un-throttles (K=8/8) while DMAs fill.
            wsb = singles.tile([P, P], bf16)
            nc.vector.memset(wsb, 0.0)
            wp = ps2p.tile([32, P], f32, tag="ps2", name="warm")
            NWARM = 18
            for i in range(NWARM):
                nc.tensor.matmul(wp, wsb[:, :32], wsb,
                                 start=(i == 0), stop=(i == NWARM - 1))

            # --- weights on the scalar HWDGE ring, in need-order: the first
            # matmul gates only on w1 chunk 0 (64KB), not the full weight set
            w1sb = singles.tile([P, KM, NF1], bf16)
            nc.scalar.dma_start(out=w1sb[:, :1], in_=w1[:, :1])
            nc.scalar.dma_start(out=w1sb[:, 1:], in_=w1[:, 1:])
            w1tsb = singles.tile([P, P], bf16)
            nc.scalar.dma_start(out=w1tsb, in_=w1t)
            b1sb = singles.tile([P, 2], f32)
            nc.scalar.dma_start(out=b1sb, in_=b1)
            w2sb = singles.tile([P, 2, NO], bf16)
            nc.scalar.dma_start(out=w2sb, in_=w2)
            b2sb = singles.tile([NO, 1], f32)
            nc.scalar.dma_start(out=b2sb, in_=b2)

            # --- x loads on the sync HWDGE ring (serialized per-ring, so
            # ring order = priority). Group 0 in 128-col quarters so the
            # opening matmuls gate on ~200KB, the rest whole-group.
            xtiles = [None] * NGRP
            xttiles = [None] * NBLK

            def load_group(g):
                xg = xp.tile([P, KM, NB], bf16, tag="x", name=f"x_{g}")
                if g == 0:
                    for q in range(4):
                        qs = slice(q * 128, (q + 1) * 128)
                        nc.sync.dma_start(out=xg[:, :, qs], in_=xm[:, g, :, qs])
                else:
                    nc.sync.dma_start(out=xg, in_=xm[:, g])
                xtiles[g] = xg

            def load_tail(b):
                xt = xtp.tile([P, NB], bf16, tag="xt", name=f"xt_{b}")
                nc.sync.dma_start(out=xt, in_=xtl[:, b])
                xttiles[b] = xt

            load_group(0)
            load_tail(0)
            load_group(1)

            def layer2(hs, g, last=False):
                gs = slice(g * NB, (g + 1) * NB)
                ps2 = ps2p.tile([NO, NB], f32, tag="ps2", name=f"ps2_{g}")
                for m in range(2):
                    nc.tensor.matmul(ps2, w2sb[:, m, :], hs[m],
                                     start=(m == 0), stop=(m == 1))
                osb = op.tile([NO, NB], f32, tag="o", name=f"o_{g}")
                nc.vector.tensor_scalar_add(osb, ps2, b2sb)
                if last:
                    # final store on the (now empty) sync ring
                    nc.sync.dma_start(out=out[:, gs], in_=osb)
                else:
                    nc.gpsimd.dma_start(out=out[:, gs], in_=osb)

            pend = []
            for blk in range(NBLK):
                g0, g1 = 2 * blk, 2 * blk + 1
                # prefetch next block's x (ring order keeps priority)
                if blk + 1 < NBLK:
                    load_tail(blk + 1)
                    load_group(2 * blk + 2)
                    load_group(2 * blk + 3)

                pss = {(g, m): ps1p.tile([P, NB], f32, tag="ps1",
                                         name=f"ps1_{g}_{m}")
                       for g in (g0, g1) for m in range(2)}

                if blk == 0:
                    # group 0 in column quarters (startup), group 1 whole
                    for q in range(4):
                        qs = slice(q * 128, (q + 1) * 128)
                        for m in range(2):
                            for k in range(KM):
                                nc.tensor.matmul(
                                    pss[(g0, m)][:, qs],
                                    w1sb[:, k, m * P:(m + 1) * P],
                                    xtiles[g0][:, k, qs],
                                    start=(q == 0 and k == 0), stop=False)
                    for m in range(2):
                        for k in range(KM):
                            nc.tensor.matmul(
                                pss[(g1, m)],
                                w1sb[:, k, m * P:(m + 1) * P],
                                xtiles[g1][:, k],
                                start=(k == 0), stop=False)
                else:
                    for m in range(2):
                        for k in range(KM):
                            for g in (g0, g1):
                                nc.tensor.matmul(
                                    pss[(g, m)],
                                    w1sb[:, k, m * P:(m + 1) * P],
                                    xtiles[g][:, k],
                                    start=(k == 0), stop=False)
                        # spread the previous block's layer-2 work into the
                        # middle of this block's dense matmul stream
                        if pend:
                            layer2(*pend.pop(0))

                # 16-row contraction tail: 4 row-tiled matmuls in distinct
                # 32-row PE groups; they execute concurrently in the array
                xt = xttiles[blk]
                for r, (g, m) in enumerate(
                        [(g0, 0), (g0, 1), (g1, 0), (g1, 1)]):
                    rs = slice(32 * r, 32 * r + KT)
                    nc.tensor.matmul(pss[(g, m)], w1tsb[rs], xt[rs],
                                     start=False, stop=True,
                                     tile_position=(32 * r, 0))

                if blk < NBLK - 1:
                    for g in (g0, g1):
                        hs = []
                        for m in range(2):
                            h = hp.tile([P, NB], bf16, tag="h",
                                        name=f"h_{g}_{m}")
                            nc.scalar.activation(h, pss[(g, m)], Relu,
                                                 bias=b1sb[:, m:m + 1])
                            hs.append(h)
                        pend.append((hs, g))
                else:
                    # last block: drain everything with a short tail. g14
                    # full-width; g15 relu split across scalar/vector.
                    hs = []
                    for m in range(2):
                        h = hp.tile([P, NB], bf16, tag="h", name=f"h_{g0}_{m}")
                        nc.scalar.activation(h, pss[(g0, m)], Relu,
                                             bias=b1sb[:, m:m + 1])
                        hs.append(h)
                    if pend:
                        layer2(*pend.pop(0))
                    layer2(hs, g0)
                    h0 = hp.tile([P, NB], bf16, tag="h", name=f"h_{g1}_0")
                    nc.scalar.activation(h0, pss[(g1, 0)], Relu,
                                         bias=b1sb[:, 0:1])
                    h1 = hp.tile([P, NB], bf16, tag="h", name=f"h_{g1}_1")
                    nc.vector.tensor_scalar(h1, pss[(g1, 1)], b1sb[:, 1:2],
                                            0.0, mybir.AluOpType.add,
                                            mybir.AluOpType.max)
                    layer2([h0, h1], g1, last=True)

    nc.compile()
    return nc


def _fold_weights(conv_w, W1):
    """W1eff[784,256] such that x @ W1eff == flatten(conv(x)) @ W1.T."""
    cw = conv_w.astype(np.float64)
    W1r = W1.astype(np.float64).reshape(NF1, 26, 26).transpose(1, 2, 0)
    W1eff = np.zeros((28, 28, NF1), np.float64)
    for dr in range(3):
        for dc in range(3):
            W1eff[dr:dr + 26, dc:dc + 26, :] += cw[dr, dc] * W1r
    return W1eff.reshape(784, NF1)


def _prep_inputs(x, conv_w, W1, b1, W2, b2):
    bf16 = ml_dtypes.bfloat16
    W1eff = _fold_weights(conv_w, W1)
    w1p = np.ascontiguousarray(
        W1eff[:768].reshape(KM, P, NF1).transpose(1, 0, 2)).astype(bf16)
    w1tp = np.zeros((P, P), np.float64)
    w1tp[0:KT] = W1eff[768:784, 0:128]
    w1tp[32:32 + KT] = W1eff[768:784, 128:256]
    w1tp[64:64 + KT] = w1tp[0:KT]
    w1tp[96:96 + KT] = w1tp[32:32 + KT]
    w1tp = w1tp.astype(bf16)
    w2p = np.ascontiguousarray(
        W2.T.astype(np.float32).reshape(2, P, NO).transpose(1, 0, 2)).astype(bf16)
    b1p = np.ascontiguousarray(b1.astype(np.float32).reshape(2, P).T)  # [P, 2]
    b2p = b2.astype(np.float32).reshape(NO, 1)

    in_maps = []
    for c in range(NCORES):
        xcT = np.ascontiguousarray(
            x[c * BC:(c + 1) * BC].T).astype(bf16)  # [784, BC]
        xmain = np.ascontiguousarray(
            xcT[:768].reshape(KM, P, NGRP, NB).transpose(1, 2, 0, 3))
        xtail = np.zeros((P, NBLK, NB), bf16)
        tl = xcT[768:784].reshape(KT, NBLK, 2, NB)
        xtail[0:KT] = tl[:, :, 0]
        xtail[32:32 + KT] = tl[:, :, 0]
        xtail[64:64 + KT] = tl[:, :, 1]
        xtail[96:96 + KT] = tl[:, :, 1]
        in_maps.append({
            "xm": xmain, "xtl": xtail,
            "w1": w1p, "w1t": w1tp, "w2": w2p, "b1": b1p, "b2": b2p,
        })
    return in_maps


def kernel(x, conv_w, W1, b1, W2, b2, _trace=False, _trace_kwargs=None):
    global _PROG
    from concourse import bass_utils

    x = np.asarray(x, dtype=np.float32)
    conv_w = np.asarray(conv_w, dtype=np.float32)
    W1 = np.asarray(W1, dtype=np.float32)
    b1 = np.asarray(b1, dtype=np.float32)
    W2 = np.asarray(W2, dtype=np.float32)
    b2 = np.asarray(b2, dtype=np.float32)
    assert x.shape == (B, 784), x.shape

    if _PROG is None:
        _PROG = _build_program()

    in_maps = _prep_inputs(x, conv_w, W1, b1, W2, b2)
    kwargs = dict(_trace_kwargs or {})
    res = bass_utils.run_bass_kernel_spmd(
        _PROG, in_maps, core_ids=list(range(NCORES)), trace=_trace, **kwargs)

    out = np.empty((B, NO), np.float32)
    for c in range(NCORES):
        out[c * BC:(c + 1) * BC] = res.results[c]["out"].T
    if _trace:
        return out, res
    return out
